# revision 2
# baseline (speedup 1.0000x reference)
"""Decoder-only transformer (GPT-style, post-LN) forward pass on 8 Trainium2 cores.

Sharding: 2 batch groups x 4 cores. Within a group, core j owns the four
128-token q-tiles {j, 7-j, 8+j, 15-j} of its batch's 2048-token sequence — a
causality-balanced assignment. K/V are all-gathered per layer within each
4-core group; attention runs a single uniform instruction stream (SPMD) in
which gathered k-tile g is matched against the suffix of local q-tiles wide
enough for every rank (width 512/384/256/128 for g 0-3/4-7/8-11/12-15);
per-core masks (host-baked, triangle included) zero invalid pairs.

AV is computed transposed (out = v^T @ es, N=512) so the attention output
lands directly in the d-major layout the wo matmul wants; the softmax
denominator rides along as a ones-row of v^T and normalization happens at
evacuation via a K=1 broadcast matmul.

The LM head is token-local over the full padded vocabulary (no final
all-gather). All matmuls are bf16 with f32 accumulation; layernorm and the
residual stream stay f32. Output logits are bf16.
"""

import math

import numpy as np
import ml_dtypes

import concourse.bass as bass
import concourse.bacc as bacc
import concourse.mybir as mybir
import concourse.tile as tile
from concourse.bass_utils import run_bass_kernel_spmd
from concourse.masks import make_identity

# model dims (hardcoded per problem spec)
V, S, D, NL, H = 50257, 2048, 768, 4, 12
HD, DF, B = 64, 3072, 2
NC = 8           # cores
CH = 512         # tokens per core
QT = 4           # 128-token q tiles per core
DT = 6           # 128-wide d tiles
FT = 24          # 128-wide dff tiles
GT = 16          # global 128-token k tiles per batch sequence
VP = 51200       # padded vocab (100 x 512)
VCH = 100        # vocab chunks of 512
RANKS = 4        # cores per batch group

F32 = mybir.dt.float32
F32R = mybir.dt.float32r
BF16 = mybir.dt.bfloat16
I32 = mybir.dt.int32
AX = mybir.AxisListType.X
OP = mybir.AluOpType
AF = mybir.ActivationFunctionType
P = 128

# uniform attention-block geometry (same for every rank)
QS = [128 * (g // RANKS) for g in range(GT)]      # first covered q column
WID = [CH - q for q in QS]                        # block width
OFFS = np.cumsum([0] + WID).tolist()              # mask column offsets
MTOT = OFFS[-1]                                   # 5120

_CACHE = {}


def owned_tiles(j):
    """Global q-tile indices owned by group-rank j (ascending)."""
    return sorted([j, 7 - j, 8 + j, 15 - j])


def g_to_rank_slot(g):
    """Which (group-rank, local slot) produced k-tile g."""
    for r in range(RANKS):
        t = owned_tiles(r)
        if g in t:
            return r, t.index(g)
    raise AssertionError


def build():
    nc = bacc.Bacc(None, target_bir_lowering=False, num_devices=NC)

    # ---- kernel I/O ----
    ids = nc.dram_tensor("ids", [P, QT], I32, kind="ExternalInput")
    pe_in = nc.dram_tensor("pe", [P, QT, D], F32, kind="ExternalInput")
    mask_in = nc.dram_tensor("masks", [P, MTOT], BF16, kind="ExternalInput")
    tok_emb = nc.dram_tensor("tok_emb", [V, D], BF16, kind="ExternalInput")
    wq_d = nc.dram_tensor("wq", [NL, D, D], BF16, kind="ExternalInput")
    wk_d = nc.dram_tensor("wk", [NL, D, D], BF16, kind="ExternalInput")
    wv_d = nc.dram_tensor("wv", [NL, D, D], BF16, kind="ExternalInput")
    wo_d = nc.dram_tensor("wo", [NL, D, D], BF16, kind="ExternalInput")
    w1_d = nc.dram_tensor("w1", [NL, D, DF], BF16, kind="ExternalInput")
    w2_d = nc.dram_tensor("w2", [NL, DF, D], BF16, kind="ExternalInput")
    b1_d = nc.dram_tensor("b1", [NL, DF], F32, kind="ExternalInput")
    b2_d = nc.dram_tensor("b2", [NL, D], F32R, kind="ExternalInput")
    ln1g_d = nc.dram_tensor("ln1_g", [NL, D], F32R, kind="ExternalInput")
    ln1b_d = nc.dram_tensor("ln1_b", [NL, D], F32R, kind="ExternalInput")
    ln2g_d = nc.dram_tensor("ln2_g", [NL, D], F32R, kind="ExternalInput")
    ln2b_d = nc.dram_tensor("ln2_b", [NL, D], F32R, kind="ExternalInput")
    lnfg_d = nc.dram_tensor("lnf_g", [1, D], F32R, kind="ExternalInput")
    lnfb_d = nc.dram_tensor("lnf_b", [1, D], F32R, kind="ExternalInput")
    lmw_d = nc.dram_tensor("lm_w", [D, VP], BF16, kind="ExternalInput")
    lmb_d = nc.dram_tensor("lm_b", [1, VP], F32R, kind="ExternalInput")
    ones_d = nc.dram_tensor("c_ones", [1, P], F32R, kind="ExternalInput")
    logits = nc.dram_tensor("logits", [CH, VP], BF16, kind="ExternalOutput")

    g4 = [[0, 1, 2, 3], [4, 5, 6, 7]]

    with tile.TileContext(nc) as tc:
        with (
            tc.tile_pool(name="pers", bufs=1) as pers,
            tc.tile_pool(name="dram", bufs=1, space="DRAM") as dram,
        ):
            h = pers.tile([P, QT, D], F32, name="h_res")
            ones_sb = pers.tile([1, P], F32R, name="ones_sb")
            nc.sync.dma_start(ones_sb[:], ones_d[:])
            id32 = pers.tile([P, P], F32, name="id32")
            make_identity(nc, id32[:])

            # ---------- embedding: gather + positional encoding ----------
            with tc.tile_pool(name="embp", bufs=1) as ep:
                ids_sb = ep.tile([P, QT], I32)
                nc.sync.dma_start(ids_sb[:], ids[:])
                pe_sb = ep.tile([P, QT, D], F32)
                nc.sync.dma_start(pe_sb[:], pe_in[:])
                for qt in range(QT):
                    emb = ep.tile([P, D], BF16, tag="emb", bufs=2)
                    nc.gpsimd.indirect_dma_start(
                        out=emb[:],
                        out_offset=None,
                        in_=tok_emb[:],
                        in_offset=bass.IndirectOffsetOnAxis(ap=ids_sb[:, qt : qt + 1], axis=0),
                    )
                    nc.vector.tensor_tensor(h[:, qt, :], emb[:], pe_sb[:, qt, :], OP.add)

            # ---------- transformer layers ----------
            with (
                tc.tile_pool(name="wk", bufs=1) as wk,
                tc.tile_pool(name="psb", bufs=1, space="PSUM") as psb,
            ):
                mask_sb = wk.tile([P, MTOT], BF16, tag="mask", name="mask_sb")
                nc.sync.dma_start(mask_sb[:], mask_in[:])
                lnp_g = wk.tile([P, D], F32, tag="lnpg", name="lnp_g")
                lnp_b = wk.tile([P, D], F32, tag="lnpb", name="lnp_b")
                scr = wk.tile([P, D], F32, tag="scr", name="scr")
                prow = wk.tile([1, D], F32R, tag="prow", name="prow")

                def bcast_row(dst, row_dram_ap):
                    """dst[p, :] = row for all p (via K=1 matmul)."""
                    nc.sync.dma_start(prow[:], row_dram_ap)
                    pb = psb.tile([P, D], F32, tag="big", bufs=2, name="pb_bcast")
                    nc.tensor.matmul(pb[:, 0:512], ones_sb[:], prow[:, 0:512], start=True, stop=True)
                    nc.tensor.matmul(pb[:, 512:D], ones_sb[:], prow[:, 512:D], start=True, stop=True)
                    nc.vector.tensor_copy(out=dst[:], in_=pb[:])

                def layernorm(g_row, b_row):
                    """in-place LN over the feature axis of h."""
                    bcast_row(lnp_g, g_row)
                    bcast_row(lnp_b, b_row)
                    for qt in range(QT):
                        x = h[:, qt, :]
                        ssum = wk.tile([P, 1], F32, tag="st1", name="ssum")
                        nc.vector.tensor_reduce(out=ssum[:], in_=x, axis=AX, op=OP.add)
                        ssq = wk.tile([P, 1], F32, tag="st2", name="ssq")
                        nc.scalar.activation(scr[:], x, AF.Square, accum_out=ssq[:])
                        mean = wk.tile([P, 1], F32, tag="st3", name="mean")
                        nc.vector.tensor_scalar_mul(mean[:], ssum[:], 1.0 / D)
                        bias_t = wk.tile([P, 1], F32, tag="st4", name="bias_t")
                        nc.vector.tensor_tensor(bias_t[:], mean[:], mean[:], OP.mult)
                        nc.vector.tensor_scalar(bias_t[:], bias_t[:], -1.0, 1e-5, OP.mult, OP.add)
                        sstd = wk.tile([P, 1], F32, tag="st5", name="sstd")
                        nc.scalar.activation(sstd[:], ssq[:], AF.Sqrt, bias=bias_t[:], scale=1.0 / D)
                        rstd = wk.tile([P, 1], F32, tag="st6", name="rstd")
                        nc.vector.reciprocal(rstd[:], sstd[:])
                        nc.vector.tensor_scalar(scr[:], x, mean[:], rstd[:], OP.subtract, OP.mult)
                        nc.vector.tensor_tensor(scr[:], scr[:], lnp_g[:], OP.mult)
                        nc.vector.tensor_tensor(h[:, qt, :], scr[:], lnp_b[:], OP.add)

                def transpose_h(dst):
                    """dst[:, dt, qt*128:...] = bf16 transpose of h's 128x128 blocks."""
                    for qt in range(QT):
                        for dt in range(DT):
                            pt = psb.tile([P, 512], F32, tag="sc", bufs=2, name="pt_tr")
                            nc.tensor.transpose(pt[:, :P], h[:, qt, dt * P : (dt + 1) * P], id32[:])
                            nc.vector.tensor_copy(out=dst[:, dt, qt * P : (qt + 1) * P], in_=pt[:, :P])

                for l in range(NL):
                    with nc.named_scope(f"layer{l}"):
                        # --- h^T (bf16) for all projections ---
                        hT = wk.tile([P, DT, CH], BF16, tag="t6", bufs=2, name=f"hT_{l}")
                        transpose_h(hT)

                        # --- K^T = (h @ wk)^T scaled by 1/sqrt(hd) ---
                        kT_w = wk.tile([P, DT, CH], BF16, tag="t6", bufs=2, name=f"kT_{l}")
                        for od in range(DT):
                            wqs = wk.tile([P, DT, P], BF16, tag="wqs", bufs=2, name="wk_c")
                            nc.sync.dma_start(
                                wqs[:],
                                wk_d[l].rearrange("(o p) f -> p o f", p=P)[:, :, od * P : (od + 1) * P],
                            )
                            ps = psb.tile([P, 512], F32, tag="sc", bufs=2, name="ps_k")
                            for kt in range(DT):
                                nc.tensor.matmul(
                                    ps[:], wqs[:, kt, :], hT[:, kt, :], start=(kt == 0), stop=(kt == DT - 1)
                                )
                            nc.vector.tensor_scalar_mul(kT_w[:, od, :], ps[:], HD ** (-0.5))
                        kt_in = dram.tile([D, CH], BF16, name=f"kt_in{l}")
                        nc.sync.dma_start(kt_in.rearrange("(o p) f -> p o f", p=P), kT_w[:])
                        kt_ag = dram.tile([RANKS * D, CH], BF16, name=f"kt_ag{l}")
                        nc.gpsimd.collective_compute(
                            "AllGather", OP.bypass, replica_groups=g4,
                            ins=[kt_in[:].opt()], outs=[kt_ag[:].opt()],
                        )

                        # --- V = h @ wv (token-major, ones column per head) ---
                        wvf = wk.tile([P, DT, D], BF16, tag="wvf", bufs=1, name="wv_f")
                        nc.sync.dma_start(wvf[:], wv_d[l].rearrange("(o p) f -> p o f", p=P))
                        v_w = wk.tile([P, QT, H, HD + 1], BF16, tag="vw", name="v_w")
                        nc.vector.memset(v_w[:, :, :, HD], 1.0)
                        for qt in range(QT):
                            pv = psb.tile([P, D], F32, tag="big", bufs=2, name="ps_v")
                            for kt in range(DT):
                                nc.tensor.matmul(
                                    pv[:, 0:512],
                                    hT[:, kt, qt * P : (qt + 1) * P],
                                    wvf[:, kt, 0:512],
                                    start=(kt == 0),
                                    stop=(kt == DT - 1),
                                )
                                nc.tensor.matmul(
                                    pv[:, 512:D],
                                    hT[:, kt, qt * P : (qt + 1) * P],
                                    wvf[:, kt, 512:D],
                                    start=(kt == 0),
                                    stop=(kt == DT - 1),
                                )
                            nc.vector.tensor_copy(
                                out=v_w[:, qt, :, 0:HD],
                                in_=pv[:].rearrange("p (h e) -> p h e", e=HD),
                            )
                        v_in = dram.tile([CH, H * (HD + 1)], BF16, name=f"v_in{l}")
                        nc.sync.dma_start(
                            v_in.rearrange("(q p) (h e) -> p q h e", p=P, e=HD + 1), v_w[:]
                        )

                        # --- all-gather K^T and V within each batch group ---
                        v_ag = dram.tile([RANKS * CH, H * (HD + 1)], BF16, name=f"v_ag{l}")
                        nc.gpsimd.collective_compute(
                            "AllGather", OP.bypass, replica_groups=g4,
                            ins=[v_in[:].opt()], outs=[v_ag[:].opt()],
                        )

                        # --- Q^T = (h @ wq)^T (overlaps the AG) ---
                        qT = wk.tile([P, DT, CH], BF16, tag="q6", name=f"qT_{l}")
                        for od in range(DT):
                            wqs = wk.tile([P, DT, P], BF16, tag="wqs", bufs=2, name="wq_c")
                            nc.sync.dma_start(
                                wqs[:],
                                wq_d[l].rearrange("(o p) f -> p o f", p=P)[:, :, od * P : (od + 1) * P],
                            )
                            ps = psb.tile([P, 512], F32, tag="sc", bufs=2, name="ps_q")
                            for kt in range(DT):
                                nc.tensor.matmul(
                                    ps[:], wqs[:, kt, :], hT[:, kt, :], start=(kt == 0), stop=(kt == DT - 1)
                                )
                            nc.vector.tensor_copy(out=qT[:, od, :], in_=ps[:])

                        # --- load gathered K^T / V ---
                        # ktg[p, r*DT+od, t]: d = od*128 + p, producer rank r, local token t
                        ktg = wk.tile([P, RANKS * DT, CH], BF16, tag="ktg", name=f"ktg_{l}")
                        nc.sync.dma_start(
                            ktg[:], kt_ag.rearrange("(c p) f -> p c f", p=P)
                        )
                        # vag[p, c, hh, e]: gathered chunk c = r*4 + slot
                        vag = wk.tile([P, GT, H, HD + 1], BF16, tag="kv24", name=f"vag_{l}")
                        nc.sync.dma_start(
                            vag[:], v_ag.rearrange("(c p) (h e) -> p c h e", p=P, e=HD + 1)
                        )

                        # --- attention: uniform causal stream, transposed AV ---
                        oT = wk.tile([P, DT, CH], BF16, tag="o6", name=f"oT_{l}")
                        for hh in range(H):
                            pb_ = (hh % 2) * 64
                            od = hh // 2
                            po = psb.tile([HD + 1, CH], F32, tag="av", bufs=2, name="po_av")
                            for g in range(GT):
                                qs, w = QS[g], WID[g]
                                r, slot = g_to_rank_slot(g)
                                ps_s = psb.tile([P, 512], F32, tag="sc", bufs=2, name="ps_s")
                                nc.tensor.matmul(
                                    ps_s[:, 0:w],
                                    ktg[pb_ : pb_ + 64, r * DT + od, slot * P : (slot + 1) * P],
                                    qT[pb_ : pb_ + 64, od, qs:CH],
                                    start=True,
                                    stop=True,
                                )
                                e = wk.tile([P, CH], BF16, tag="es", bufs=5, name="es")
                                nc.scalar.activation(e[:, 0:w], ps_s[:, 0:w], AF.Exp)
                                nc.vector.tensor_tensor(
                                    e[:, 0:w], e[:, 0:w],
                                    mask_sb[:, OFFS[g] : OFFS[g] + w], OP.mult,
                                )
                                nc.tensor.matmul(
                                    po[:, qs:CH],
                                    vag[:, r * QT + slot, hh, :],
                                    e[:, 0:w],
                                    start=(g == 0),
                                    stop=(g == GT - 1),
                                )
                            # normalize by the ones-row sums and evacuate into oT
                            rec = wk.tile([1, CH], F32R, tag="rec", bufs=2, name="rec")
                            with nc.allow_low_precision(reason="f32r holds f32 bits"):
                                nc.vector.reciprocal(rec[:], po[HD : HD + 1, :])
                            prb = psb.tile([P, 512], F32, tag="sc", bufs=2, name="prb")
                            nc.tensor.matmul(prb[0:64, :], ones_sb[:, 0:64], rec[:], start=True, stop=True)
                            rbc = wk.tile([64, CH], F32, tag="rbc", bufs=2, name="rbc")
                            nc.scalar.activation(rbc[:], prb[0:64, :], AF.Copy)
                            nc.vector.tensor_tensor(
                                oT[pb_ : pb_ + 64, od, :], po[0:HD, :], rbc[:], OP.mult
                            )

                        # --- mha = O @ wo, residual, LN1 ---
                        wof = wk.tile([P, DT, D], BF16, tag="wvf", bufs=1, name="wo_f")
                        nc.sync.dma_start(wof[:], wo_d[l].rearrange("(o p) f -> p o f", p=P))
                        for qt in range(QT):
                            pm = psb.tile([P, D], F32, tag="big", bufs=2, name="ps_wo")
                            for kt in range(DT):
                                nc.tensor.matmul(
                                    pm[:, 0:512],
                                    oT[:, kt, qt * P : (qt + 1) * P],
                                    wof[:, kt, 0:512],
                                    start=(kt == 0),
                                    stop=(kt == DT - 1),
                                )
                                nc.tensor.matmul(
                                    pm[:, 512:D],
                                    oT[:, kt, qt * P : (qt + 1) * P],
                                    wof[:, kt, 512:D],
                                    start=(kt == 0),
                                    stop=(kt == DT - 1),
                                )
                            nc.vector.tensor_tensor(h[:, qt, :], h[:, qt, :], pm[:], OP.add)
                        layernorm(ln1g_d[l : l + 1, :], ln1b_d[l : l + 1, :])

                        # --- FFN (w2 SBUF-resident; single pass over w1) ---
                        hT2 = wk.tile([P, DT, CH], BF16, tag="t6", bufs=2, name=f"hT2_{l}")
                        transpose_h(hT2)
                        w2f = wk.tile([P, FT, D], BF16, tag="w2f", name="w2_f")
                        nc.sync.dma_start(w2f[:], w2_d[l].rearrange("(o p) f -> p o f", p=P))
                        b1_sb = wk.tile([P, FT], F32, tag="b1s", name="b1_sb")
                        nc.sync.dma_start(b1_sb[:], b1_d[l : l + 1, :].rearrange("a (o p) -> p (a o)", p=P))
                        b2_sb = wk.tile([1, D], F32R, tag="b2s", name="b2_sb")
                        nc.sync.dma_start(b2_sb[:], b2_d[l : l + 1, :])
                        f1c = wk.tile([P, FT, CH], BF16, tag="kv24", name="f1c")
                        for df in range(FT):
                            w1c = wk.tile([P, DT, P], BF16, tag="w1s", bufs=2, name="w1c")
                            nc.sync.dma_start(
                                w1c[:],
                                w1_d[l].rearrange("(o p) f -> p o f", p=P)[:, :, df * P : (df + 1) * P],
                            )
                            pf1 = psb.tile([P, 512], F32, tag="sc", bufs=2, name="ps_f1")
                            for kt in range(DT):
                                nc.tensor.matmul(
                                    pf1[:],
                                    w1c[:, kt, :],
                                    hT2[:, kt, :],
                                    start=(kt == 0),
                                    stop=(kt == DT - 1),
                                )
                            nc.scalar.activation(
                                f1c[:, df, :], pf1[:], AF.Relu, bias=b1_sb[:, df : df + 1]
                            )
                        for qp in range(2):
                            pf2 = [None, None]
                            for qtl in range(2):
                                pf2[qtl] = psb.tile([P, D], F32, tag="big", bufs=2, name="ps_f2")
                                nc.tensor.matmul(pf2[qtl][:, 0:512], ones_sb[:], b2_sb[:, 0:512], start=True, stop=False)
                                nc.tensor.matmul(pf2[qtl][:, 512:D], ones_sb[:], b2_sb[:, 512:D], start=True, stop=False)
                            for df in range(FT):
                                for qtl in range(2):
                                    qt = qp * 2 + qtl
                                    nc.tensor.matmul(
                                        pf2[qtl][:, 0:512],
                                        f1c[:, df, qt * P : (qt + 1) * P],
                                        w2f[:, df, 0:512],
                                        start=False,
                                        stop=(df == FT - 1),
                                    )
                                    nc.tensor.matmul(
                                        pf2[qtl][:, 512:D],
                                        f1c[:, df, qt * P : (qt + 1) * P],
                                        w2f[:, df, 512:D],
                                        start=False,
                                        stop=(df == FT - 1),
                                    )
                            for qtl in range(2):
                                qt = qp * 2 + qtl
                                nc.vector.tensor_tensor(h[:, qt, :], h[:, qt, :], pf2[qtl][:], OP.add)
                        layernorm(ln2g_d[l : l + 1, :], ln2b_d[l : l + 1, :])

                # ---------- final LN + LM head (token-local) ----------
                with nc.named_scope("final"):
                    layernorm(lnfg_d[:], lnfb_d[:])
                    hTf = wk.tile([P, DT, CH], BF16, tag="t6", bufs=2, name="hTf")
                    transpose_h(hTf)

                with nc.named_scope("lmhead"):
                    for n in range(VCH):
                        lmw_c = wk.tile([P, DT, 512], BF16, tag="lmw", bufs=2, name="lmw_c")
                        nc.sync.dma_start(
                            lmw_c[:],
                            lmw_d.rearrange("(o p) f -> p o f", p=P)[:, :, n * 512 : (n + 1) * 512],
                        )
                        lmb_c = wk.tile([1, 512], F32R, tag="lmb", bufs=2, name="lmb_c")
                        nc.sync.dma_start(lmb_c[:], lmb_d[:, n * 512 : (n + 1) * 512])
                        pbias = psb.tile([P, 512], F32, tag="av", bufs=2, name="ps_lmb")
                        nc.tensor.matmul(pbias[:], ones_sb[:], lmb_c[:], start=True, stop=True)
                        bias_sb = wk.tile([P, 512], BF16, tag="lmbb", bufs=2, name="bias_sb")
                        nc.scalar.activation(bias_sb[:], pbias[:], AF.Copy)
                        for qt in range(QT):
                            po = psb.tile([P, 512], F32, tag="sc", bufs=2, name="ps_lm")
                            for dt in range(DT):
                                nc.tensor.matmul(
                                    po[:],
                                    hTf[:, dt, qt * P : (qt + 1) * P],
                                    lmw_c[:, dt, :],
                                    start=(dt == 0),
                                    stop=(dt == DT - 1),
                                )
                            osb = wk.tile([P, 512], BF16, tag="osb", bufs=4, name="o_sb")
                            nc.vector.tensor_tensor(osb[:], po[:], bias_sb[:], OP.add)
                            nc.sync.dma_start(
                                logits[qt * P : (qt + 1) * P, n * 512 : (n + 1) * 512],
                                osb[:],
                            )

    return _finish(nc)


def _finish(nc):
    nc.compile()
    return nc


def _pe_table():
    pos = np.arange(S, dtype=np.float32)[:, None]
    div = np.exp(np.arange(0, D, 2, dtype=np.float32) * (-math.log(10000.0) / D))
    pe = np.zeros((S, D), dtype=np.float32)
    pe[:, 0::2] = np.sin(pos * div)
    pe[:, 1::2] = np.cos(pos * div)
    return pe


def _masks_for_rank(j):
    """[128, MTOT] bf16: for each k-tile g and covered q column, 1 if
    k_global <= q_global (includes the diagonal triangle), else 0."""
    tiles = owned_tiles(j)
    m = np.zeros((P, MTOT), dtype=np.float32)
    for g in range(GT):
        qs = QS[g]
        k_global = g * P + np.arange(P)[:, None]                      # [128,1]
        cols = np.arange(qs, CH)                                      # local q cols
        q_global = np.array([tiles[c // P] * P + (c % P) for c in cols])[None, :]
        m[:, OFFS[g] : OFFS[g] + WID[g]] = (k_global <= q_global)
    return m.astype(ml_dtypes.bfloat16)


def kernel(**inputs):
    if "nc" not in _CACHE:
        _CACHE["nc"] = build()
    nc = _CACHE["nc"]

    bf = lambda a: np.ascontiguousarray(np.asarray(a, dtype=np.float32).astype(ml_dtypes.bfloat16))
    f32 = lambda a: np.ascontiguousarray(np.asarray(a), dtype=np.float32)

    x = np.asarray(inputs["x"])
    wq = bf(np.asarray(inputs["wq"], dtype=np.float32).transpose(0, 2, 1, 3).reshape(NL, D, D))
    wk_ = bf(np.asarray(inputs["wk"], dtype=np.float32).transpose(0, 2, 1, 3).reshape(NL, D, D))
    wv = bf(np.asarray(inputs["wv"], dtype=np.float32).transpose(0, 2, 1, 3).reshape(NL, D, D))
    pe = _pe_table()

    lmw_pad = np.zeros((D, VP), dtype=ml_dtypes.bfloat16)
    lmw_pad[:, :V] = bf(inputs["lm_w"])
    lmb_pad = np.zeros((1, VP), dtype=np.float32)
    lmb_pad[0, :V] = f32(inputs["lm_b"])

    common = {
        "tok_emb": bf(inputs["tok_emb"]),
        "wq": wq, "wk": wk_, "wv": wv,
        "wo": bf(inputs["wo"]), "w1": bf(inputs["w1"]), "w2": bf(inputs["w2"]),
        "b1": f32(inputs["b1"]), "b2": f32(inputs["b2"]),
        "ln1_g": f32(inputs["ln1_g"]), "ln1_b": f32(inputs["ln1_b"]),
        "ln2_g": f32(inputs["ln2_g"]), "ln2_b": f32(inputs["ln2_b"]),
        "lnf_g": f32(inputs["lnf_g"]).reshape(1, D),
        "lnf_b": f32(inputs["lnf_b"]).reshape(1, D),
        "lm_w": lmw_pad, "lm_b": lmb_pad,
        "c_ones": np.ones((1, P), dtype=np.float32),
    }

    rank_masks = [_masks_for_rank(j) for j in range(RANKS)]
    in_maps = []
    for c in range(NC):
        b, j = c // RANKS, c % RANKS
        tiles = owned_tiles(j)
        toks = np.concatenate([x[b, g * P : (g + 1) * P] for g in tiles]).astype(np.int32)
        ids_c = toks.reshape(QT, P).T.copy()  # [128, 4]
        pe_c = np.stack([pe[g * P : (g + 1) * P] for g in tiles], axis=0)  # [4,128,768]
        pe_c = np.ascontiguousarray(pe_c.transpose(1, 0, 2))  # [128,4,768]
        in_maps.append({**common, "ids": ids_c, "pe": pe_c, "masks": rank_masks[j]})

    res = run_bass_kernel_spmd(nc, in_maps, core_ids=list(range(NC)))
    _CACHE["last_result"] = res
    _CACHE["last_in_maps"] = in_maps

    out = np.zeros((B, S, V), dtype=np.float32)
    for c in range(NC):
        b, j = c // RANKS, c % RANKS
        tiles = owned_tiles(j)
        lg = np.asarray(res.results[c]["logits"], dtype=np.float32)  # [512, VP]
        for i, g in enumerate(tiles):
            out[b, g * P : (g + 1) * P, :] = lg[i * P : (i + 1) * P, :V]
    return out


if __name__ == "__main__":
    import time

    t0 = time.time()
    nc = build()
    print(f"build ok: {time.time() - t0:.1f}s")
